# Initial kernel scaffold
#
"""GCMC layer kernel for 8 Trainium2 NeuronCores.

Strategy (per direction, users and items independently):
  - Shard destination nodes contiguously across the 8 cores.
  - Sort edges by (core, dst-block of 128 nodes); pad each block's edge count
    to a multiple of 128, uniform across cores (SPMD: one program, 8 cores).
  - Device: per 128-edge tile, indirect-DMA gather of concatenated
    [W_row | cj] 65-float rows; multiply by cj (tensor_scalar); build a
    one-hot selection matrix from dst labels (is_equal vs iota); matmul
    lhsT=selection rhs=messages accumulating the block's segment-sum in PSUM.
  - Per block: x = ci * psum; leaky = max(x, 0.1x); transpose; FC matmul
    (+bias); DMA out.
Outputs are block-major per core; host reassembles the full arrays.
"""
import numpy as np
import ml_dtypes

R, E = 5, 500000
NU, NI = 100000, 50000
D, OUT = 64, 64
NCORES = 8
P = 128
ROWW = D + 1  # gathered row: 64 features + cj


def _prep_direction(dst, src, ridx, src_n, dst_n):
    """Host-side edge sharding/sorting/padding for one direction.

    Returns per-core gather-index and label arrays (partition-major) plus the
    uniform tiles-per-block array.
    """
    per_core = dst_n // NCORES
    nblk = (per_core + P - 1) // P
    core = dst // per_core
    local = dst - core * per_core
    block = local >> 7
    label = (local & 127).astype(np.float32)
    gidx = (ridx * src_n + src).astype(np.int32)
    zero_row = R * src_n  # appended all-zeros table row for padding

    key = core.astype(np.int64) * nblk + block
    order = np.argsort(key, kind="stable")
    key_s = key[order]
    cnt = np.bincount(key_s, minlength=NCORES * nblk).reshape(NCORES, nblk)
    tpb = np.maximum(1, -(-cnt.max(axis=0) // P))  # ceil, >=1
    T = int(tpb.sum())
    blk_slot0 = np.zeros(nblk + 1, dtype=np.int64)
    np.cumsum(tpb * P, out=blk_slot0[1:])

    # position of each sorted edge within its (core, block) group
    n = key_s.shape[0]
    starts = np.zeros(NCORES * nblk + 1, dtype=np.int64)
    np.cumsum(cnt.reshape(-1), out=starts[1:])
    pos = np.arange(n, dtype=np.int64) - starts[key_s]

    widx = np.full((NCORES, T * P), zero_row, dtype=np.int32)
    lab = np.zeros((NCORES, T * P), dtype=np.float32)
    cs = core[order]
    slot = blk_slot0[block[order]] + pos
    widx[cs, slot] = gidx[order]
    lab[cs, slot] = label[order]
    # [core, T*P] -> [core, 128, T] partition-major (slot = t*128 + p)
    widx = np.ascontiguousarray(widx.reshape(NCORES, T, P).transpose(0, 2, 1))
    lab = np.ascontiguousarray(
        lab.reshape(NCORES, T, P).transpose(0, 2, 1)
    ).astype(ml_dtypes.bfloat16)
    return widx, lab, tpb.astype(np.int64), nblk


def _prep_ci(ci, dst_n):
    per_core = dst_n // NCORES
    nblk = (per_core + P - 1) // P
    out = np.zeros((NCORES, nblk * P), np.float32)
    out[:, :per_core] = ci.reshape(NCORES, per_core)
    # [core, nblk*P] with node index B*128+p -> [core, 128, nblk]
    return np.ascontiguousarray(out.reshape(NCORES, nblk, P).transpose(0, 2, 1))


def _build_nc(tpb_u, tpb_i, nblk_u, nblk_i, tu_rows, ti_rows):
    import concourse.bass as bass
    import concourse.bacc as bacc
    import concourse.mybir as mybir
    import concourse.tile as tile
    from concourse.masks import make_identity

    Tu, Ti = int(tpb_u.sum()), int(tpb_i.sum())
    maxt = int(max(tpb_u.max(), tpb_i.max()))
    f32, bf16, i32 = mybir.dt.float32, mybir.dt.bfloat16, mybir.dt.int32

    nc = bacc.Bacc(None, target_bir_lowering=False)
    t_tab_u = nc.dram_tensor("tab_u", [tu_rows, ROWW], f32, kind="ExternalInput")
    t_tab_i = nc.dram_tensor("tab_i", [ti_rows, ROWW], f32, kind="ExternalInput")
    t_widx_u = nc.dram_tensor("widx_u", [P, Tu], i32, kind="ExternalInput")
    t_widx_i = nc.dram_tensor("widx_i", [P, Ti], i32, kind="ExternalInput")
    t_lab_u = nc.dram_tensor("lab_u", [P, Tu], bf16, kind="ExternalInput")
    t_lab_i = nc.dram_tensor("lab_i", [P, Ti], bf16, kind="ExternalInput")
    t_ci_u = nc.dram_tensor("ci_u", [P, nblk_u], f32, kind="ExternalInput")
    t_ci_i = nc.dram_tensor("ci_i", [P, nblk_i], f32, kind="ExternalInput")
    t_fcw_u = nc.dram_tensor("fcw_u", [D, OUT], f32, kind="ExternalInput")
    t_fcw_i = nc.dram_tensor("fcw_i", [D, OUT], f32, kind="ExternalInput")
    t_b_u = nc.dram_tensor("b_u", [P, OUT], f32, kind="ExternalInput")
    t_b_i = nc.dram_tensor("b_i", [P, OUT], f32, kind="ExternalInput")
    t_iota = nc.dram_tensor("iota", [P, P], bf16, kind="ExternalInput")
    t_out_u = nc.dram_tensor("out_u", [nblk_u * P, OUT], f32, kind="ExternalOutput")
    t_out_i = nc.dram_tensor("out_i", [nblk_i * P, OUT], f32, kind="ExternalOutput")

    with tile.TileContext(nc) as tc:
        with (
            tc.tile_pool(name="const", bufs=1) as cp,
            tc.tile_pool(name="grp", bufs=2) as gp,
            tc.tile_pool(name="post", bufs=3) as pp,
            tc.tile_pool(name="acc", bufs=4, space="PSUM") as accp,
            tc.tile_pool(name="psmisc", bufs=2, space="PSUM") as tpp,
            tc.tile_pool(name="psfc", bufs=2, space="PSUM") as fcp,
        ):
            iota_t = cp.tile([P, P], bf16, tag="iota")
            nc.sync.dma_start(out=iota_t[:], in_=t_iota[:])
            ident_t = cp.tile([P, P], f32, tag="ident")
            make_identity(nc, ident_t[:])
            fcw_u_t = cp.tile([D, OUT], f32, tag="fcwu")
            nc.sync.dma_start(out=fcw_u_t[:], in_=t_fcw_u[:])
            fcw_i_t = cp.tile([D, OUT], f32, tag="fcwi")
            nc.sync.dma_start(out=fcw_i_t[:], in_=t_fcw_i[:])
            b_u_t = cp.tile([P, OUT], f32, tag="bu")
            nc.sync.dma_start(out=b_u_t[:], in_=t_b_u[:])
            b_i_t = cp.tile([P, OUT], f32, tag="bi")
            nc.sync.dma_start(out=b_i_t[:], in_=t_b_i[:])
            ci_u_t = cp.tile([P, nblk_u], f32, tag="ciu")
            nc.sync.dma_start(out=ci_u_t[:], in_=t_ci_u[:])
            ci_i_t = cp.tile([P, nblk_i], f32, tag="cii")
            nc.sync.dma_start(out=ci_i_t[:], in_=t_ci_i[:])
            widx_u_t = cp.tile([P, Tu], i32, tag="wxu")
            nc.sync.dma_start(out=widx_u_t[:], in_=t_widx_u[:])
            widx_i_t = cp.tile([P, Ti], i32, tag="wxi")
            nc.sync.dma_start(out=widx_i_t[:], in_=t_widx_i[:])
            lab_u_t = cp.tile([P, Tu], bf16, tag="lbu")
            nc.sync.dma_start(out=lab_u_t[:], in_=t_lab_u[:])
            lab_i_t = cp.tile([P, Ti], bf16, tag="lbi")
            nc.sync.dma_start(out=lab_i_t[:], in_=t_lab_i[:])

            for (tpb, nblk, tab, widx_t, lab_t, ci_t, fcw_t, b_t, t_out) in (
                (tpb_u, nblk_u, t_tab_i, widx_u_t, lab_u_t, ci_u_t, fcw_u_t, b_u_t, t_out_u),
                (tpb_i, nblk_i, t_tab_u, widx_i_t, lab_i_t, ci_i_t, fcw_i_t, b_i_t, t_out_i),
            ):
                g0 = 0
                for B in range(nblk):
                    T = int(tpb[B])
                    raw = gp.tile([P, maxt * ROWW], f32, tag="raw")
                    for t in range(T):
                        nc.gpsimd.indirect_dma_start(
                            out=raw[:, t * ROWW:(t + 1) * ROWW],
                            out_offset=None,
                            in_=tab[:],
                            in_offset=bass.IndirectOffsetOnAxis(
                                ap=widx_t[:, g0 + t:g0 + t + 1], axis=0),
                        )
                    msgb = gp.tile([P, maxt * D], bf16, tag="msgb")
                    nc.vector.tensor_tensor(
                        out=msgb[:, :T * D].rearrange("p (t d) -> p t d", d=D),
                        in0=raw[:, :T * ROWW].rearrange(
                            "p (t w) -> p t w", w=ROWW)[:, :, 0:D],
                        in1=raw[:, :T * ROWW].rearrange(
                            "p (t w) -> p t w", w=ROWW)[:, :, D:ROWW].to_broadcast(
                                [P, T, D]),
                        op=mybir.AluOpType.mult,
                    )
                    sel = gp.tile([P, maxt * P], bf16, tag="sel")
                    nc.vector.tensor_tensor(
                        out=sel[:, :T * P].rearrange("p (t q) -> p t q", q=P),
                        in0=lab_t[:, g0:g0 + T].rearrange(
                            "p (t o) -> p t o", o=1).to_broadcast([P, T, P]),
                        in1=iota_t[:].rearrange(
                            "p (o q) -> p o q", o=1).to_broadcast([P, T, P]),
                        op=mybir.AluOpType.is_equal,
                    )
                    acc = accp.tile([P, D], f32, tag="acc", space="PSUM")
                    for t in range(T):
                        nc.tensor.matmul(
                            out=acc[:],
                            lhsT=sel[:, t * P:(t + 1) * P],
                            rhs=msgb[:, t * D:(t + 1) * D],
                            start=(t == 0),
                            stop=(t == T - 1),
                        )
                    # x = ci * segsum ; leaky = max(x, 0.1 x)
                    y = pp.tile([P, D], f32, tag="y")
                    nc.vector.tensor_scalar(
                        out=y[:], in0=acc[:], scalar1=ci_t[:, B:B + 1],
                        scalar2=None, op0=mybir.AluOpType.mult)
                    xb = pp.tile([P, D], f32, tag="xb")
                    nc.vector.tensor_scalar(
                        out=xb[:], in0=y[:], scalar1=0.1, scalar2=None,
                        op0=mybir.AluOpType.mult)
                    nc.vector.tensor_tensor(
                        out=xb[:], in0=y[:], in1=xb[:], op=mybir.AluOpType.max)
                    # transpose -> [D, P]
                    pt = tpp.tile([D, P], f32, tag="pt", space="PSUM")
                    nc.tensor.transpose(out=pt[:], in_=xb[:], identity=ident_t[:])
                    xT = pp.tile([D, P], f32, tag="xT")
                    nc.vector.tensor_copy(out=xT[:], in_=pt[:])
                    fo = fcp.tile([P, OUT], f32, tag="fo", space="PSUM")
                    nc.tensor.matmul(out=fo[:], lhsT=xT[:], rhs=fcw_t[:],
                                     start=True, stop=True)
                    ob = pp.tile([P, OUT], f32, tag="ob")
                    nc.vector.tensor_tensor(
                        out=ob[:], in0=fo[:], in1=b_t[:], op=mybir.AluOpType.add)
                    nc.sync.dma_start(
                        out=t_out[B * P:(B + 1) * P, :], in_=ob[:])
                    g0 += T
    nc.compile()
    return nc


def _run(nc, in_maps):
    from concourse.bass_utils import run_bass_kernel_spmd
    res = run_bass_kernel_spmd(nc, in_maps, list(range(NCORES)), trace=False)
    return res.results


def kernel(edge_u, edge_i, cj_u, ci_u, cj_i, ci_i,
           W_user, W_item, ufc_w, ufc_b, ifc_w, ifc_b):
    edge_u = np.asarray(edge_u); edge_i = np.asarray(edge_i)
    cj_u = np.asarray(cj_u, np.float32); ci_u = np.asarray(ci_u, np.float32)
    cj_i = np.asarray(cj_i, np.float32); ci_i = np.asarray(ci_i, np.float32)
    W_user = np.asarray(W_user, np.float32); W_item = np.asarray(W_item, np.float32)
    ufc_w = np.asarray(ufc_w, np.float32); ufc_b = np.asarray(ufc_b, np.float32)
    ifc_w = np.asarray(ifc_w, np.float32); ifc_b = np.asarray(ifc_b, np.float32)

    r, e = edge_u.shape
    ridx = np.broadcast_to(np.arange(r, dtype=np.int64)[:, None], (r, e)).ravel()
    du = edge_u.astype(np.int64).ravel()
    di = edge_i.astype(np.int64).ravel()

    # users direction: dst=user, gathers W_item/cj_i rows
    widx_u, lab_u, tpb_u, nblk_u = _prep_direction(du, di, ridx, NI, NU)
    # items direction: dst=item, gathers W_user/cj_u rows
    widx_i, lab_i, tpb_i, nblk_i = _prep_direction(di, du, ridx, NU, NI)

    # concat tables [W | cj] with trailing zero row
    tab_u = np.zeros((r * NU + 1, ROWW), np.float32)  # gathered by items-dir
    tab_u[:r * NU, :D] = W_user.reshape(r * NU, D)
    tab_u[:r * NU, D] = np.tile(cj_u.ravel(), r)
    tab_i = np.zeros((r * NI + 1, ROWW), np.float32)  # gathered by users-dir
    tab_i[:r * NI, :D] = W_item.reshape(r * NI, D)
    tab_i[:r * NI, D] = np.tile(cj_i.ravel(), r)

    ci_u_arr = _prep_ci(ci_u.ravel(), NU)
    ci_i_arr = _prep_ci(ci_i.ravel(), NI)
    iota = np.ascontiguousarray(
        np.broadcast_to(np.arange(P, dtype=np.float32), (P, P))
    ).astype(ml_dtypes.bfloat16)
    b_u = np.broadcast_to(ufc_b, (P, OUT)).copy()
    b_i = np.broadcast_to(ifc_b, (P, OUT)).copy()

    nc = _build_nc(tpb_u, tpb_i, nblk_u, nblk_i, tab_u.shape[0], tab_i.shape[0])
    in_maps = []
    for k in range(NCORES):
        in_maps.append({
            "tab_u": tab_u, "tab_i": tab_i,
            "widx_u": widx_u[k], "widx_i": widx_i[k],
            "lab_u": lab_u[k], "lab_i": lab_i[k],
            "ci_u": ci_u_arr[k], "ci_i": ci_i_arr[k],
            "fcw_u": ufc_w, "fcw_i": ifc_w,
            "b_u": b_u, "b_i": b_i,
            "iota": iota,
        })
    results = _run(nc, in_maps)
    per_u, per_i = NU // NCORES, NI // NCORES
    ufeat = np.concatenate([results[k]["out_u"][:per_u] for k in range(NCORES)])
    ifeat = np.concatenate([results[k]["out_i"][:per_i] for k in range(NCORES)])
    return ufeat, ifeat


# revision 3
# speedup vs baseline: 1.3940x; 1.3940x over previous
"""GCMC layer kernel for 8 Trainium2 NeuronCores.

Strategy (per direction, users and items independently):
  - Shard destination nodes contiguously across the 8 cores.
  - Sort edges by (core, dst-block of 128 nodes); pad each block's edge count
    to a multiple of 128, uniform across cores (SPMD: one program, 8 cores).
  - Device: per 128-edge tile, indirect-DMA gather of concatenated
    [W_row | cj] 65-float rows; multiply by cj (tensor_scalar); build a
    one-hot selection matrix from dst labels (is_equal vs iota); matmul
    lhsT=selection rhs=messages accumulating the block's segment-sum in PSUM.
  - Per block: x = ci * psum; leaky = max(x, 0.1x); transpose; FC matmul
    (+bias); DMA out.
Outputs are block-major per core; host reassembles the full arrays.
"""
import numpy as np
import ml_dtypes

R, E = 5, 500000
NU, NI = 100000, 50000
D, OUT = 64, 64
NCORES = 8
P = 128
ROWW = D + 1  # gathered row: 64 features + cj


def _prep_direction(dst, src, ridx, src_n, dst_n, r):
    """Host-side edge sharding/sorting/padding for one direction.

    Returns per-core gather-index and label arrays (partition-major) plus the
    uniform tiles-per-block array.
    """
    per_core = dst_n // NCORES
    nblk = (per_core + P - 1) // P
    core = dst // per_core
    local = dst - core * per_core
    block = local >> 7
    label = (local & 127).astype(np.float32)
    gidx = (ridx * src_n + src).astype(np.int32)
    zero_row = r * src_n  # appended all-zeros table row for padding

    key = core.astype(np.int64) * nblk + block
    order = np.argsort(key, kind="stable")
    key_s = key[order]
    cnt = np.bincount(key_s, minlength=NCORES * nblk).reshape(NCORES, nblk)
    tpb = np.maximum(1, -(-cnt.max(axis=0) // P))  # ceil, >=1
    T = int(tpb.sum())
    blk_slot0 = np.zeros(nblk + 1, dtype=np.int64)
    np.cumsum(tpb * P, out=blk_slot0[1:])

    # position of each sorted edge within its (core, block) group
    n = key_s.shape[0]
    starts = np.zeros(NCORES * nblk + 1, dtype=np.int64)
    np.cumsum(cnt.reshape(-1), out=starts[1:])
    pos = np.arange(n, dtype=np.int64) - starts[key_s]

    widx = np.full((NCORES, T * P), zero_row, dtype=np.int32)
    lab = np.zeros((NCORES, T * P), dtype=np.float32)
    cs = core[order]
    slot = blk_slot0[block[order]] + pos
    widx[cs, slot] = gidx[order]
    lab[cs, slot] = label[order]
    # [core, T*P] -> [core, 128, T] partition-major (slot = t*128 + p)
    widx = np.ascontiguousarray(widx.reshape(NCORES, T, P).transpose(0, 2, 1))
    lab = np.ascontiguousarray(
        lab.reshape(NCORES, T, P).transpose(0, 2, 1)
    ).astype(ml_dtypes.bfloat16)
    return widx, lab, tpb.astype(np.int64), nblk


def _prep_ci(ci, dst_n):
    per_core = dst_n // NCORES
    nblk = (per_core + P - 1) // P
    out = np.zeros((NCORES, nblk * P), np.float32)
    out[:, :per_core] = ci.reshape(NCORES, per_core)
    # [core, nblk*P] with node index B*128+p -> [core, 128, nblk]
    return np.ascontiguousarray(out.reshape(NCORES, nblk, P).transpose(0, 2, 1))


def _build_nc(tpb_u, tpb_i, nblk_u, nblk_i, tu_rows, ti_rows):
    import concourse.bass as bass
    import concourse.bacc as bacc
    import concourse.mybir as mybir
    import concourse.tile as tile
    from concourse.masks import make_identity

    Tu, Ti = int(tpb_u.sum()), int(tpb_i.sum())
    maxt = int(max(tpb_u.max(), tpb_i.max()))
    f32, bf16, i32 = mybir.dt.float32, mybir.dt.bfloat16, mybir.dt.int32

    nc = bacc.Bacc(None, target_bir_lowering=False)
    t_tab_u = nc.dram_tensor("tab_u", [tu_rows, ROWW], f32, kind="ExternalInput")
    t_tab_i = nc.dram_tensor("tab_i", [ti_rows, ROWW], f32, kind="ExternalInput")
    t_widx_u = nc.dram_tensor("widx_u", [P, Tu], i32, kind="ExternalInput")
    t_widx_i = nc.dram_tensor("widx_i", [P, Ti], i32, kind="ExternalInput")
    t_lab_u = nc.dram_tensor("lab_u", [P, Tu], bf16, kind="ExternalInput")
    t_lab_i = nc.dram_tensor("lab_i", [P, Ti], bf16, kind="ExternalInput")
    t_ci_u = nc.dram_tensor("ci_u", [P, nblk_u], f32, kind="ExternalInput")
    t_ci_i = nc.dram_tensor("ci_i", [P, nblk_i], f32, kind="ExternalInput")
    t_fcw_u = nc.dram_tensor("fcw_u", [D, OUT], f32, kind="ExternalInput")
    t_fcw_i = nc.dram_tensor("fcw_i", [D, OUT], f32, kind="ExternalInput")
    t_b_u = nc.dram_tensor("b_u", [P, OUT], f32, kind="ExternalInput")
    t_b_i = nc.dram_tensor("b_i", [P, OUT], f32, kind="ExternalInput")
    t_iota = nc.dram_tensor("iota", [P, P], bf16, kind="ExternalInput")
    t_out_u = nc.dram_tensor("out_u", [nblk_u * P, OUT], f32, kind="ExternalOutput")
    t_out_i = nc.dram_tensor("out_i", [nblk_i * P, OUT], f32, kind="ExternalOutput")

    with tile.TileContext(nc) as tc:
        with (
            tc.tile_pool(name="const", bufs=1) as cp,
            tc.tile_pool(name="grp", bufs=2) as gp,
            tc.tile_pool(name="post", bufs=3) as pp,
            tc.tile_pool(name="acc", bufs=4, space="PSUM") as accp,
            tc.tile_pool(name="psmisc", bufs=2, space="PSUM") as tpp,
            tc.tile_pool(name="psfc", bufs=2, space="PSUM") as fcp,
        ):
            iota_t = cp.tile([P, P], bf16, tag="iota")
            nc.sync.dma_start(out=iota_t[:], in_=t_iota[:])
            ident_t = cp.tile([P, P], f32, tag="ident")
            make_identity(nc, ident_t[:])
            fcw_u_t = cp.tile([D, OUT], f32, tag="fcwu")
            nc.sync.dma_start(out=fcw_u_t[:], in_=t_fcw_u[:])
            fcw_i_t = cp.tile([D, OUT], f32, tag="fcwi")
            nc.sync.dma_start(out=fcw_i_t[:], in_=t_fcw_i[:])
            b_u_t = cp.tile([P, OUT], f32, tag="bu")
            nc.sync.dma_start(out=b_u_t[:], in_=t_b_u[:])
            b_i_t = cp.tile([P, OUT], f32, tag="bi")
            nc.sync.dma_start(out=b_i_t[:], in_=t_b_i[:])
            ci_u_t = cp.tile([P, nblk_u], f32, tag="ciu")
            nc.sync.dma_start(out=ci_u_t[:], in_=t_ci_u[:])
            ci_i_t = cp.tile([P, nblk_i], f32, tag="cii")
            nc.sync.dma_start(out=ci_i_t[:], in_=t_ci_i[:])
            widx_u_t = cp.tile([P, Tu], i32, tag="wxu")
            nc.sync.dma_start(out=widx_u_t[:], in_=t_widx_u[:])
            widx_i_t = cp.tile([P, Ti], i32, tag="wxi")
            nc.sync.dma_start(out=widx_i_t[:], in_=t_widx_i[:])
            lab_u_t = cp.tile([P, Tu], bf16, tag="lbu")
            nc.sync.dma_start(out=lab_u_t[:], in_=t_lab_u[:])
            lab_i_t = cp.tile([P, Ti], bf16, tag="lbi")
            nc.sync.dma_start(out=lab_i_t[:], in_=t_lab_i[:])

            for (tpb, nblk, tab, widx_t, lab_t, ci_t, fcw_t, b_t, t_out) in (
                (tpb_u, nblk_u, t_tab_i, widx_u_t, lab_u_t, ci_u_t, fcw_u_t, b_u_t, t_out_u),
                (tpb_i, nblk_i, t_tab_u, widx_i_t, lab_i_t, ci_i_t, fcw_i_t, b_i_t, t_out_i),
            ):
                g0 = 0
                for B in range(nblk):
                    T = int(tpb[B])
                    raw = gp.tile([P, maxt * ROWW], f32, tag="raw")
                    for t in range(T):
                        nc.gpsimd.indirect_dma_start(
                            out=raw[:, t * ROWW:(t + 1) * ROWW],
                            out_offset=None,
                            in_=tab[:],
                            in_offset=bass.IndirectOffsetOnAxis(
                                ap=widx_t[:, g0 + t:g0 + t + 1], axis=0),
                        )
                    msgb = gp.tile([P, maxt * D], bf16, tag="msgb")
                    nc.vector.tensor_tensor(
                        out=msgb[:, :T * D].rearrange("p (t d) -> p t d", d=D),
                        in0=raw[:, :T * ROWW].rearrange(
                            "p (t w) -> p t w", w=ROWW)[:, :, 0:D],
                        in1=raw[:, :T * ROWW].rearrange(
                            "p (t w) -> p t w", w=ROWW)[:, :, D:ROWW].to_broadcast(
                                [P, T, D]),
                        op=mybir.AluOpType.mult,
                    )
                    sel = gp.tile([P, maxt * P], bf16, tag="sel")
                    nc.vector.tensor_tensor(
                        out=sel[:, :T * P].rearrange("p (t q) -> p t q", q=P),
                        in0=lab_t[:, g0:g0 + T].rearrange(
                            "p (t o) -> p t o", o=1).to_broadcast([P, T, P]),
                        in1=iota_t[:].rearrange(
                            "p (o q) -> p o q", o=1).to_broadcast([P, T, P]),
                        op=mybir.AluOpType.is_equal,
                    )
                    acc = accp.tile([P, D], f32, tag="acc", space="PSUM")
                    for t in range(T):
                        nc.tensor.matmul(
                            out=acc[:],
                            lhsT=sel[:, t * P:(t + 1) * P],
                            rhs=msgb[:, t * D:(t + 1) * D],
                            start=(t == 0),
                            stop=(t == T - 1),
                        )
                    # x = ci * segsum ; leaky = max(x, 0.1 x)
                    y = pp.tile([P, D], f32, tag="y")
                    nc.vector.tensor_scalar(
                        out=y[:], in0=acc[:], scalar1=ci_t[:, B:B + 1],
                        scalar2=None, op0=mybir.AluOpType.mult)
                    xb = pp.tile([P, D], f32, tag="xb")
                    nc.vector.tensor_scalar(
                        out=xb[:], in0=y[:], scalar1=0.1, scalar2=None,
                        op0=mybir.AluOpType.mult)
                    nc.vector.tensor_tensor(
                        out=xb[:], in0=y[:], in1=xb[:], op=mybir.AluOpType.max)
                    # transpose -> [D, P]
                    pt = tpp.tile([D, P], f32, tag="pt", space="PSUM")
                    nc.tensor.transpose(out=pt[:], in_=xb[:], identity=ident_t[:])
                    xT = pp.tile([D, P], f32, tag="xT")
                    nc.vector.tensor_copy(out=xT[:], in_=pt[:])
                    fo = fcp.tile([P, OUT], f32, tag="fo", space="PSUM")
                    nc.tensor.matmul(out=fo[:], lhsT=xT[:], rhs=fcw_t[:],
                                     start=True, stop=True)
                    ob = pp.tile([P, OUT], f32, tag="ob")
                    nc.vector.tensor_tensor(
                        out=ob[:], in0=fo[:], in1=b_t[:], op=mybir.AluOpType.add)
                    nc.sync.dma_start(
                        out=t_out[B * P:(B + 1) * P, :], in_=ob[:])
                    g0 += T
    nc.compile()
    return nc


def _run(nc, in_maps):
    from concourse.bass_utils import run_bass_kernel_spmd
    res = run_bass_kernel_spmd(nc, in_maps, list(range(NCORES)), trace=False)
    return res.results


def prepare(edge_u, edge_i, cj_u, ci_u, cj_i, ci_i,
            W_user, W_item, ufc_w, ufc_b, ifc_w, ifc_b):
    edge_u = np.asarray(edge_u); edge_i = np.asarray(edge_i)
    cj_u = np.asarray(cj_u, np.float32); ci_u = np.asarray(ci_u, np.float32)
    cj_i = np.asarray(cj_i, np.float32); ci_i = np.asarray(ci_i, np.float32)
    W_user = np.asarray(W_user, np.float32); W_item = np.asarray(W_item, np.float32)
    ufc_w = np.asarray(ufc_w, np.float32); ufc_b = np.asarray(ufc_b, np.float32)
    ifc_w = np.asarray(ifc_w, np.float32); ifc_b = np.asarray(ifc_b, np.float32)

    r, e = edge_u.shape
    ridx = np.broadcast_to(np.arange(r, dtype=np.int64)[:, None], (r, e)).ravel()
    du = edge_u.astype(np.int64).ravel()
    di = edge_i.astype(np.int64).ravel()

    # users direction: dst=user, gathers W_item/cj_i rows
    widx_u, lab_u, tpb_u, nblk_u = _prep_direction(du, di, ridx, NI, NU, r)
    # items direction: dst=item, gathers W_user/cj_u rows
    widx_i, lab_i, tpb_i, nblk_i = _prep_direction(di, du, ridx, NU, NI, r)

    # concat tables [W | cj] with trailing zero row
    tab_u = np.zeros((r * NU + 1, ROWW), np.float32)  # gathered by items-dir
    tab_u[:r * NU, :D] = W_user.reshape(r * NU, D)
    tab_u[:r * NU, D] = np.tile(cj_u.ravel(), r)
    tab_i = np.zeros((r * NI + 1, ROWW), np.float32)  # gathered by users-dir
    tab_i[:r * NI, :D] = W_item.reshape(r * NI, D)
    tab_i[:r * NI, D] = np.tile(cj_i.ravel(), r)

    ci_u_arr = _prep_ci(ci_u.ravel(), NU)
    ci_i_arr = _prep_ci(ci_i.ravel(), NI)
    iota = np.ascontiguousarray(
        np.broadcast_to(np.arange(P, dtype=np.float32), (P, P))
    ).astype(ml_dtypes.bfloat16)
    b_u = np.broadcast_to(ufc_b, (P, OUT)).copy()
    b_i = np.broadcast_to(ifc_b, (P, OUT)).copy()

    nc = _build_nc(tpb_u, tpb_i, nblk_u, nblk_i, tab_u.shape[0], tab_i.shape[0])
    in_maps = []
    for k in range(NCORES):
        in_maps.append({
            "tab_u": tab_u, "tab_i": tab_i,
            "widx_u": widx_u[k], "widx_i": widx_i[k],
            "lab_u": lab_u[k], "lab_i": lab_i[k],
            "ci_u": ci_u_arr[k], "ci_i": ci_i_arr[k],
            "fcw_u": ufc_w, "fcw_i": ifc_w,
            "b_u": b_u, "b_i": b_i,
            "iota": iota,
        })
    return nc, in_maps


def _assemble(results):
    per_u, per_i = NU // NCORES, NI // NCORES
    ufeat = np.concatenate([results[k]["out_u"][:per_u] for k in range(NCORES)])
    ifeat = np.concatenate([results[k]["out_i"][:per_i] for k in range(NCORES)])
    return ufeat, ifeat


def kernel(**inputs):
    nc, in_maps = prepare(**inputs)
    return _assemble(_run(nc, in_maps))
